# revision 11
# baseline (speedup 1.0000x reference)
"""Trainium2 Bass kernel for nn_LossNet_42494406426743 (contrastive loss_fn).

Math (reference, temp=0.1, B=4096):
    xn = l2_normalize(x); xe, ye, ze = split(xn, 3)
    For pairs (a,b) in {xx, yy, xy, xz, yz(+transposes zx, zy)}:
        d_ab[i] = exp(a_i.b_i/t)  (diagonal)
        s_ab[i] = sum_j exp(a_i.b_j/t)  (row sums of the exp-similarity matrix)
    loss = mean_{ij}[-2 log(d_xy[j]/((S[i]-D[j])))] + 4 aux terms of
           mean_{ij}[-log(d[j]/(s[i]-d[j]))]

Device work (sharded 8 ways over rows; each core owns 256 "low" + 256 "high"
rows of each of xe and ye; z never appears as a row operand).  The exp +
row-sum work (4.5*B^2/8 elements per core) is spread over FOUR engines:

  * TensorE: bf16 matmuls (stationary own-row chunks vs the SBUF-resident
    embedding matrix) into two PSUM rings, plus ones-matmul partition
    reductions of the column accumulators.
  * ScalarE (ACT ring, 2x1536 PSUM): exact exp via LUT with fused accum_out
    row-sums.
  * GpSimdE (GPS ring, 2x512 PSUM): evacuates the other matmul outputs to
    SBUF as bf16 logits (otherwise idle engine).
  * VectorE: approximate exp on the GPS-path logits via the fp16 bit trick
    -- tensor_scalar int16(logit*14773.13 + 15301.5) runs at 4x mode from
    bf16, and a second 4x tensor_scalar over the fp16-bitcast tile yields
    the row sum through accum_out.  Per-element error <4% and zero-mean
    (offset calibrated), so row sums over 4096 terms are accurate to ~1e-4.
    VectorE also accumulates the exp tiles of XZ^T / YZ^T (and the
    symmetric-block right halves) into column accumulators, whose
    partition sums recover the zx / zy row sums and the xx / yy high-row
    left halves without recomputing transposed exps.

Host work (O(B), fp64): diagonals, assembling s vectors, and the
mean_{ij} log(s[i]-d[j]) terms evaluated exactly via a binomial power-series
factorization (O(B*K) instead of O(B^2); exact fallback if out of range).
"""

import numpy as np
import ml_dtypes

_BF16 = ml_dtypes.bfloat16

# Problem constants (hardcoded per harness contract).
_N = 12288          # total rows
_D = 128            # feature dim
_B = 4096           # rows per split
_NCORES = 8
_TEMP = 0.1
_EPS = 1e-12

# fp16 bit-trick exp constants: int16(logit*EXPA + EXPB) viewed as fp16
# approximates exp(logit/temp).  EXPB includes a -58.5 offset that nulls the
# mean relative error of the linear-mantissa approximation.
_EXPA = 1024.0 * (1.0 / _TEMP) * np.log2(np.e)   # 14773.13
_EXPB = 15.0 * 1024.0 - 58.5

_STATE = {}

# --------------------------------------------------------------------------
# Static work plan.
# m-chunks: m0,m1 = "low" x rows, m2,m3 = "high" x rows, m4,m5 = low y,
# m6,m7 = high y (128 rows each).  Low rows compute their symmetric block
# fully; high rows compute only the right half and recover the left half
# from transposed colsums (xxB / yyB).
#
# Subregions per m-chunk (uniform target vector and colacc-ness), emitted
# z-first so the column accumulators complete early:
#   m0/m1: XZ[8192:12288|zx] XX-L[0:2048] xxB[2048:4096|xxB] XY[4096:8192]
#   m2/m3: XZ[8192:12288|zx] XX-R[2048:4096] XY[4096:8192]
#   m4/m5: YZ[8192:12288|zy] YY-L[4096:6144] yyB[6144:8192|yyB]
#   m6/m7: YZ[8192:12288|zy] YY-R[6144:8192]
_COLACC_SHAPE = {"zx": _B, "zy": _B, "xxB": 2048, "yyB": 2048}
_COLACC_BASE = {"zx": 8192, "zy": 8192, "xxB": 2048, "yyB": 6144}
_COLACC_FIRST_M = {"zx": 0, "zy": 4, "xxB": 0, "yyB": 4}
_CS_LAYOUT = {"zx": (0, 32), "xxB": (32, 16), "yyB": (48, 16), "zy": (64, 32)}
_CS_NCH = 96


def _subregions(m):
    if m < 2:
        return [(8192, 4096, "ax", "zx"), (0, 2048, "xx", None),
                (2048, 2048, "xx", "xxB"), (4096, 4096, "xy", None)]
    if m < 4:
        return [(8192, 4096, "ax", "zx"), (2048, 2048, "xx", None),
                (4096, 4096, "xy", None)]
    if m < 6:
        return [(8192, 4096, "ay", "zy"), (4096, 2048, "yy", None),
                (6144, 2048, "yy", "yyB")]
    return [(8192, 4096, "ay", "zy"), (6144, 2048, "yy", None)]


# 2048-aligned chunks routed through the DVE trick-exp path (PSUM convert
# at 1x + fp16 row-sum at 4x); the rest take the ACT path.  Tuned so the
# ACT and DVE engine busies balance (~30.7K vs ~43K columns).
_DVE_CHUNKS = set()
for _m in range(4):
    _DVE_CHUNKS.add((_m, 4096))     # XY first half
    _DVE_CHUNKS.add((_m, 6144))     # XY second half
_DVE_CHUNKS.add((3, 2048))          # XX-R (m3)
for _m in (6, 7):
    _DVE_CHUNKS.add((_m, 6144))     # YY-R
for _m in (4, 5, 6, 7):
    _DVE_CHUNKS.add((_m, 8192))     # YZ first half

# Colacc adds routed to GpSimd (real HW rate ~2.0 ns/col vs 0.55 on
# VectorE): only a minority, scheduled early so the reduce-matmuls that
# consume the colaccs never wait on a GpSimd backlog.  Keyed by (m, cskey).
_GPS_ADD_KEYS = {(1, "zx"), (1, "xxB"), (5, "zy"), (5, "yyB")}


def _act_tiling(width, first=False):
    """Split an ACT span into instruction widths (512-multiples)."""
    if first:
        # smaller leading instructions cut the startup bubble
        return [512, 512, 1024] + _act_tiling(width - 2048) if width > 2048 \
            else [512] * (width // 512)
    out = []
    while width:
        if width >= 2048:
            out.append(1536)
            width -= 1536
            if width == 512:
                out.append(512)
                width = 0
        else:
            out.append(min(width, 1536))
            width -= out[-1]
    return out


def _make_plan():
    """Emission-ordered op list per m-chunk.  Each entry:
    ('act', m, col0, w, slot, cls, cskey) or ('gps', m, col0, 2048, slot,
    cls, cskey).  Slot indices follow emission order."""
    plan = []
    slot = 0
    for m in range(8):
        ops = []
        for (r0, rw, cls, cskey) in _subregions(m):
            # split into GPS chunks and leftover ACT runs
            runs = []
            c = r0
            while c < r0 + rw:
                if (m, c) in _DVE_CHUNKS:
                    runs.append(("dve", c, 2048))
                    c += 2048
                else:
                    # extend an act run
                    if runs and runs[-1][0] == "act":
                        runs[-1] = ("act", runs[-1][1], runs[-1][2] + 2048)
                    else:
                        runs.append(("act", c, 2048))
                    c += 2048
            for kind, c0, w in runs:
                if kind == "dve":
                    ops.append(["dve", m, c0, 2048, None, cls, cskey])
                else:
                    for tw in _act_tiling(w):
                        ops.append(["act", m, c0, tw, None, cls, cskey])
                        c0 += tw
        for op in ops:
            op[4] = slot
            slot += 1
        plan.append([tuple(o) for o in ops])
    return plan, slot


_PLAN, _NSLOT = _make_plan()

# Reduce-emission points: after which m-chunk to emit each colacc's
# partition-sum matmuls (colacc writers: xxB m0-1, zx m0-3, yyB m4-5,
# zy m4-7).
_REDUCE_AFTER_M = {4: ["xxB"], 5: ["zx"], 7: ["yyB", "zy"]}


def _build_nc(T=1):
    import concourse.bacc as bacc
    import concourse.mybir as mybir
    import concourse.tile as tile

    f32 = mybir.dt.float32
    bf16 = mybir.dt.bfloat16

    nc = bacc.Bacc("TRN2")
    lhsT = nc.dram_tensor("lhsT", [128, 1024], bf16, kind="ExternalInput")
    rhsT = nc.dram_tensor("rhsT", [128, _N], bf16, kind="ExternalInput")
    out_s = nc.dram_tensor("out_s", [128, _NSLOT], f32, kind="ExternalOutput")
    out_cs = nc.dram_tensor("out_cs", [128, _CS_NCH], f32, kind="ExternalOutput")

    with tile.TileContext(nc) as tc:
        with (
            tc.tile_pool(name="singles", bufs=1) as singles,
            tc.tile_pool(name="etp", bufs=6) as etp,
            tc.tile_pool(name="i16p", bufs=4) as i16p,
            tc.tile_pool(name="et2p", bufs=4) as et2p,
            tc.tile_pool(name="psa", bufs=2, space="PSUM") as psa,
            tc.tile_pool(name="psd", bufs=2, space="PSUM") as psd,
        ):
            lhsT_t = singles.tile([128, 1024], bf16)
            rhsT_t = singles.tile([128, _N], bf16)
            ones_t = singles.tile([128, 1], bf16)
            act_warm = singles.tile([128, 1], f32)
            s_acc = singles.tile([128, _NSLOT], f32)
            colaccs = {k: singles.tile([128, w], bf16, name=f"colacc_{k}")
                       for k, w in _COLACC_SHAPE.items()}
            cs_sbuf = singles.tile([128, _CS_NCH], f32)

            nc.vector.memset(ones_t[:], 1.0)
            # Pull the exp ACT-table load into the input-DMA shadow.
            nc.scalar.activation(act_warm[:], ones_t[:],
                                 mybir.ActivationFunctionType.Exp, scale=1.0)
            # lhsT rides the GPSIMD SWDGE queue so it lands in parallel with
            # the rhs stream on the SP HWDGE queue.
            nc.gpsimd.dma_start(lhsT_t[:, 0:128], lhsT[:, 0:128])
            nc.sync.dma_start(rhsT_t[:, 0:1024], rhsT[:, 0:1024])
            nc.gpsimd.dma_start(lhsT_t[:, 128:1024], lhsT[:, 128:1024])
            nc.sync.dma_start(rhsT_t[:, 1024:2048], rhsT[:, 1024:2048])
            for p in range(1, _N // 2048):
                nc.sync.dma_start(rhsT_t[:, p * 2048:(p + 1) * 2048],
                                  rhsT[:, p * 2048:(p + 1) * 2048])

            for _t in range(T):
                _emit_body(nc, tc, etp, i16p, et2p, psa, psd, lhsT_t,
                           rhsT_t, ones_t, s_acc, colaccs, cs_sbuf, _t)

            nc.sync.dma_start(out_s[:], s_acc[:])
            nc.sync.dma_start(out_cs[:], cs_sbuf[:])

    nc.finalize()
    return nc


def _emit_body(nc, tc, etp, i16p, et2p, psa, psd, lhsT_t, rhsT_t,
               ones_t, s_acc, colaccs, cs_sbuf, t):
    import concourse.mybir as mybir

    f32 = mybir.dt.float32
    bf16 = mybir.dt.bfloat16
    fp16 = mybir.dt.float16
    i16 = mybir.dt.int16
    Exp = mybir.ActivationFunctionType.Exp
    mult = mybir.AluOpType.mult
    add = mybir.AluOpType.add

    def emit_reduce(key):
        base, nch = _CS_LAYOUT[key]
        cs_ps = psd.tile([128, 512], f32, tag="d", name=f"csps_{key}_{t}")
        for ch in range(nch):
            nc.tensor.matmul(
                cs_ps[:, ch:ch + 1],
                colaccs[key][:, ch * 128:(ch + 1) * 128],
                ones_t[:],
                start=True, stop=True,
            )
        nc.vector.tensor_copy(cs_sbuf[:, base:base + nch], cs_ps[:, 0:nch])

    for m, ops in enumerate(_PLAN):
        lhs_chunk = lhsT_t[:, m * 128:(m + 1) * 128]
        for (kind, _m, col0, width, slot, cls, cskey) in ops:
            first_writer = (cskey is not None
                            and m == _COLACC_FIRST_M[cskey])
            if kind == "act":
                pt = psa.tile([128, 1536], f32, tag="a",
                              name=f"pa_{t}_{m}_{col0}")
                for k in range(width // 512):
                    nc.tensor.matmul(
                        pt[:, k * 512:(k + 1) * 512],
                        lhs_chunk,
                        rhsT_t[:, col0 + k * 512:col0 + (k + 1) * 512],
                        start=True, stop=True,
                    )
                if first_writer:
                    off = col0 - _COLACC_BASE[cskey]
                    dst = colaccs[cskey][:, off:off + width]
                else:
                    et = etp.tile([128, 1536], bf16, tag="et",
                                  name=f"et_{t}_{m}_{col0}")
                    dst = et[:, 0:width]
                nc.scalar.activation(
                    dst, pt[:, 0:width], Exp, scale=1.0 / _TEMP,
                    accum_out=s_acc[:, slot:slot + 1],
                )
                if cskey is not None and not first_writer:
                    off = col0 - _COLACC_BASE[cskey]
                    ca = colaccs[cskey][:, off:off + width]
                    eng = (nc.gpsimd if (m, cskey) in _GPS_ADD_KEYS
                           else nc.vector)
                    eng.tensor_add(ca, ca, dst)
            else:
                # DVE path: int16 exp-bit convert straight from PSUM (1x),
                # then the 4x-mode fp16 row-sum tensor_scalar.
                it = i16p.tile([128, 2048], i16, tag="i16",
                               name=f"it_{t}_{m}_{col0}")
                for k in range(4):
                    pg = psd.tile([128, 512], f32, tag="d",
                                  name=f"pg_{t}_{m}_{col0}_{k}")
                    nc.tensor.matmul(
                        pg[:],
                        lhs_chunk,
                        rhsT_t[:, col0 + k * 512:col0 + (k + 1) * 512],
                        start=True, stop=True,
                    )
                    nc.vector.tensor_scalar(it[:, k * 512:(k + 1) * 512],
                                            pg[:], float(_EXPA),
                                            float(_EXPB), mult, add)
                if first_writer:
                    off = col0 - _COLACC_BASE[cskey]
                    dst = colaccs[cskey][:, off:off + width]
                else:
                    et2 = et2p.tile([128, 2048], bf16, tag="et2",
                                    name=f"e2_{t}_{m}_{col0}")
                    dst = et2[:]
                nc.vector.tensor_scalar(dst, it[:].bitcast(fp16), 1.0, None,
                                        mult, add,
                                        accum_out=s_acc[:, slot:slot + 1])
                if cskey is not None and not first_writer:
                    off = col0 - _COLACC_BASE[cskey]
                    ca = colaccs[cskey][:, off:off + width]
                    eng = (nc.gpsimd if (m, cskey) in _GPS_ADD_KEYS
                           else nc.vector)
                    eng.tensor_add(ca, ca, dst)
        for key in _REDUCE_AFTER_M.get(m, ()):
            emit_reduce(key)


class _Exec:
    """Cached sharded-jit executor for the finalized Bass module (modeled on
    concourse.bass2jax.run_bass_via_pjrt, but reusable across calls)."""

    def __init__(self, nc, n_cores):
        import jax
        import concourse.mybir as mybir
        from concourse import bass2jax
        from jax.sharding import Mesh, PartitionSpec
        from jax.experimental.shard_map import shard_map

        bass2jax.install_neuronx_cc_hook()
        self._jax = jax
        self.nc = nc
        self.n_cores = n_cores
        partition_name = (
            nc.partition_id_tensor.name if nc.partition_id_tensor else None
        )
        in_names, out_names, out_avals, zero_outs = [], [], [], []
        for alloc in nc.m.functions[0].allocations:
            if not isinstance(alloc, mybir.MemoryLocationSet):
                continue
            name = alloc.memorylocations[0].name
            if alloc.kind == "ExternalInput":
                if name != partition_name:
                    in_names.append(name)
            elif alloc.kind == "ExternalOutput":
                shape = tuple(alloc.tensor_shape)
                dtype = mybir.dt.np(alloc.dtype)
                out_names.append(name)
                out_avals.append(jax.core.ShapedArray(shape, dtype))
                zero_outs.append(np.zeros(shape, dtype))
        self.in_names = list(in_names)
        self.out_names = out_names
        self.out_avals = out_avals
        self.zero_outs = zero_outs
        n_params = len(in_names)
        n_outs = len(out_names)
        bind_in_names = in_names + out_names + (
            [partition_name] if partition_name else []
        )

        def _body(*args):
            operands = list(args)
            if partition_name is not None:
                operands.append(bass2jax.partition_id_tensor())
            outs = bass2jax._bass_exec_p.bind(
                *operands,
                out_avals=tuple(out_avals),
                in_names=tuple(bind_in_names),
                out_names=tuple(out_names),
                lowering_input_output_aliases=(),
                sim_require_finite=True,
                sim_require_nnan=True,
                nc=nc,
            )
            return tuple(outs)

        devices = jax.devices()[:n_cores]
        assert len(devices) == n_cores
        self.mesh = Mesh(np.asarray(devices), ("core",))
        donate = tuple(range(n_params, n_params + n_outs))
        self.fn = jax.jit(
            shard_map(
                _body,
                mesh=self.mesh,
                in_specs=(PartitionSpec("core"),) * (n_params + n_outs),
                out_specs=(PartitionSpec("core"),) * n_outs,
                check_rep=False,
            ),
            donate_argnums=donate,
            keep_unused=True,
        )

    def make_zeros(self):
        return [
            np.zeros((self.n_cores * z.shape[0], *z.shape[1:]), z.dtype)
            for z in self.zero_outs
        ]

    def concat_inputs(self, in_maps):
        return [
            np.concatenate([np.asarray(in_maps[c][n]) for c in range(self.n_cores)], axis=0)
            for n in self.in_names
        ]

    def run_raw(self, concat_in, zeros):
        return self.fn(*concat_in, *zeros)

    def __call__(self, in_maps):
        out_arrs = self.fn(*self.concat_inputs(in_maps), *self.make_zeros())
        res = []
        for c in range(self.n_cores):
            res.append({
                name: np.asarray(out_arrs[i]).reshape(
                    self.n_cores, *self.out_avals[i].shape)[c]
                for i, name in enumerate(self.out_names)
            })
        return res


def _get_exec(T=1):
    key = ("exec", T)
    if key not in _STATE:
        nc = _build_nc(T)
        _STATE[key] = _Exec(nc, _NCORES)
    return _STATE[key]


def _mlod_exact(s, d):
    """mean_{ij} log(s[i] - d[j]) computed directly (chunked)."""
    tot = 0.0
    for i0 in range(0, s.shape[0], 256):
        tot += float(np.log(np.subtract.outer(s[i0:i0 + 256], d)).sum())
    return tot / (s.shape[0] * d.shape[0])


def _mlod(s, d):
    """mean_{ij} log(s[i] - d[j]) via binomial power-series factorization.

    log(s_i - d_j) = log M + log1p(u_i - v_j) with M = mean(s) - mean(d),
    u = (s-mean(s))/M, v = (d-mean(d))/M.  mean_{ij} (u_i-v_j)^k factorizes
    into products of power means, so the double mean is O(B*K).
    """
    from math import comb

    s = np.asarray(s, np.float64)
    d = np.asarray(d, np.float64)
    ms, md = s.mean(), d.mean()
    M = ms - md
    if not np.isfinite(M) or M <= 0:
        return _mlod_exact(s, d)
    u = (s - ms) / M
    v = (d - md) / M
    wmax = np.abs(u).max() + np.abs(v).max()
    if wmax > 0.5:
        return _mlod_exact(s, d)
    K = 120
    P = np.empty(K + 1)
    Q = np.empty(K + 1)
    up = np.ones_like(u)
    vp = np.ones_like(v)
    for k in range(K + 1):
        P[k] = up.mean()
        Q[k] = vp.mean()
        up *= u
        vp *= -v
    total = 0.0
    for k in range(1, K + 1):
        mk = 0.0
        for m in range(k + 1):
            mk += comb(k, m) * P[m] * Q[k - m]
        term = (1.0 if k % 2 == 1 else -1.0) / k * mk
        total += term
        if k > 6 and abs(term) < 1e-18 * max(1.0, abs(total)):
            break
    return float(np.log(M)) + total


def _host_prepare(x):
    """fp32 normalize (mirrors reference), bf16 cast, per-core device inputs."""
    x = np.asarray(x, np.float32)
    n = np.sqrt((x * x).sum(axis=1, keepdims=True))
    xn = x / np.maximum(n, _EPS)
    xnb = xn.astype(_BF16)
    rhsT = np.ascontiguousarray(xnb.T)  # [128, 12288]
    H = _B // 2
    in_maps = []
    for c in range(_NCORES):
        lo = c * 256
        rows = np.concatenate([
            xnb[lo:lo + 256],                    # low x  (m0, m1)
            xnb[H + lo:H + lo + 256],            # high x (m2, m3)
            xnb[_B + lo:_B + lo + 256],          # low y  (m4, m5)
            xnb[_B + H + lo:_B + H + lo + 256],  # high y (m6, m7)
        ], axis=0)
        in_maps.append({"lhsT": np.ascontiguousarray(rows.T), "rhsT": rhsT})
    return xn, in_maps


_TARGET_VEC = {"xx": 0, "xy": 1, "ax": 2, "yy": 3, "ay": 4}


def _assemble_s(results):
    """Decode device outputs into the seven s vectors (fp64)."""
    H = _B // 2
    vecs = [np.zeros(_B) for _ in range(5)]  # xx, xy, ax, yy, ay
    s_zx = np.zeros(_B)
    s_zy = np.zeros(_B)
    for c in range(_NCORES):
        sa = np.asarray(results[c]["out_s"], np.float64)  # [128, NSLOT]
        for m, ops in enumerate(_PLAN):
            half = (m // 2) % 2            # 0 = low rows, 1 = high rows
            i0 = half * H + c * 256 + (m % 2) * 128
            for (kind, _m, col0, width, slot, cls, cskey) in ops:
                vecs[_TARGET_VEC[cls]][i0:i0 + 128] += sa[:, slot]
    # Column-sum contributions.
    cs_sum = np.zeros((128, _CS_NCH), np.float64)
    for c in range(_NCORES):
        cs_sum += np.asarray(results[c]["out_cs"], np.float64)
    # col idx base+ch holds colsums for accumulator column ch*128 + p
    s_zx += cs_sum[:, 0:32].T.reshape(-1)
    vecs[0][H:] += cs_sum[:, 32:48].T.reshape(-1)   # xx high-left
    vecs[3][H:] += cs_sum[:, 48:64].T.reshape(-1)   # yy high-left
    s_zy += cs_sum[:, 64:96].T.reshape(-1)
    s_xx, s_xy, s_ax, s_yy, s_ay = vecs
    return s_xx, s_xy, s_ax, s_yy, s_ay, s_zx, s_zy


def _host_combine(xn, results):
    xe = xn[:_B].astype(np.float64)
    ye = xn[_B:2 * _B].astype(np.float64)
    ze = xn[2 * _B:].astype(np.float64)
    inv_t = 1.0 / _TEMP
    d_xx = np.exp((xe * xe).sum(1) * inv_t)
    d_yy = np.exp((ye * ye).sum(1) * inv_t)
    d_xy = np.exp((xe * ye).sum(1) * inv_t)
    d_ax = np.exp((xe * ze).sum(1) * inv_t)
    d_ay = np.exp((ye * ze).sum(1) * inv_t)

    s_xx, s_xy, s_ax, s_yy, s_ay, s_zx, s_zy = _assemble_s(results)

    S_mut = s_xy + s_xx + s_yy
    D_mut = d_xy + d_xx + d_yy
    loss_mutual = -2.0 * float(np.log(d_xy).mean()) + 2.0 * _mlod(S_mut, D_mut)

    def aux(d, s):
        return -float(np.log(d).mean()) + _mlod(s, d)

    loss = (loss_mutual + aux(d_ax, s_ax) + aux(d_ay, s_ay)
            + aux(d_ax, s_zx) + aux(d_ay, s_zy))
    return np.array(loss, dtype=np.float32)


def kernel(x):
    ex = _get_exec()
    xn, in_maps = _host_prepare(x)
    results = ex(in_maps)
    return _host_combine(xn, results)


if __name__ == "__main__":
    rng = np.random.default_rng(0)
    x = rng.standard_normal((_N, _D)).astype(np.float32)
    print(kernel(x))


# revision 12
# speedup vs baseline: 1.1380x; 1.1380x over previous
"""Trainium2 Bass kernel for nn_LossNet_42494406426743 (contrastive loss_fn).

Math (reference, temp=0.1, B=4096):
    xn = l2_normalize(x); xe, ye, ze = split(xn, 3)
    For pairs (a,b) in {xx, yy, xy, xz, yz(+transposes zx, zy)}:
        d_ab[i] = exp(a_i.b_i/t)  (diagonal)
        s_ab[i] = sum_j exp(a_i.b_j/t)  (row sums of the exp-similarity matrix)
    loss = mean_{ij}[-2 log(d_xy[j]/((S[i]-D[j])))] + 4 aux terms of
           mean_{ij}[-log(d[j]/(s[i]-d[j]))]

Device work (sharded 8 ways over rows; each core owns 256 "low" + 256 "high"
rows of each of xe and ye; z never appears as a row operand).  The exp +
row-sum work (4.5*B^2/8 elements per core) is spread over FOUR engines:

  * TensorE: bf16 matmuls (stationary own-row chunks vs the SBUF-resident
    embedding matrix) into two PSUM rings, plus ones-matmul partition
    reductions of the column accumulators.
  * ScalarE (ACT ring, 2x1536 PSUM): exact exp via LUT with fused accum_out
    row-sums.
  * GpSimdE (GPS ring, 2x512 PSUM): evacuates the other matmul outputs to
    SBUF as bf16 logits (otherwise idle engine).
  * VectorE: approximate exp on the GPS-path logits via the fp16 bit trick
    -- tensor_scalar int16(logit*14773.13 + 15301.5) runs at 4x mode from
    bf16, and a second 4x tensor_scalar over the fp16-bitcast tile yields
    the row sum through accum_out.  Per-element error <4% and zero-mean
    (offset calibrated), so row sums over 4096 terms are accurate to ~1e-4.
    VectorE also accumulates the exp tiles of XZ^T / YZ^T (and the
    symmetric-block right halves) into column accumulators, whose
    partition sums recover the zx / zy row sums and the xx / yy high-row
    left halves without recomputing transposed exps.

Host work (O(B), fp64): diagonals, assembling s vectors, and the
mean_{ij} log(s[i]-d[j]) terms evaluated exactly via a binomial power-series
factorization (O(B*K) instead of O(B^2); exact fallback if out of range).
"""

import numpy as np
import ml_dtypes

_BF16 = ml_dtypes.bfloat16

# Problem constants (hardcoded per harness contract).
_N = 12288          # total rows
_D = 128            # feature dim
_B = 4096           # rows per split
_NCORES = 8
_TEMP = 0.1
_EPS = 1e-12

# fp16 bit-trick exp constants: int16(logit*EXPA + EXPB) viewed as fp16
# approximates exp(logit/temp).  EXPB includes a -58.5 offset that nulls the
# mean relative error of the linear-mantissa approximation.
_EXPA = 1024.0 * (1.0 / _TEMP) * np.log2(np.e)   # 14773.13
_EXPB = 15.0 * 1024.0 - 58.5

_STATE = {}

# --------------------------------------------------------------------------
# Static work plan.
# m-chunks: m0,m1 = "low" x rows, m2,m3 = "high" x rows, m4,m5 = low y,
# m6,m7 = high y (128 rows each).  Low rows compute their symmetric block
# fully; high rows compute only the right half and recover the left half
# from transposed colsums (xxB / yyB).
#
# Subregions per m-chunk (uniform target vector and colacc-ness), emitted
# z-first so the column accumulators complete early:
#   m0/m1: XZ[8192:12288|zx] XX-L[0:2048] xxB[2048:4096|xxB] XY[4096:8192]
#   m2/m3: XZ[8192:12288|zx] XX-R[2048:4096] XY[4096:8192]
#   m4/m5: YZ[8192:12288|zy] YY-L[4096:6144] yyB[6144:8192|yyB]
#   m6/m7: YZ[8192:12288|zy] YY-R[6144:8192]
_COLACC_SHAPE = {"zx": _B, "zy": _B, "xxB": 2048, "yyB": 2048}
_COLACC_BASE = {"zx": 8192, "zy": 8192, "xxB": 2048, "yyB": 6144}
_COLACC_FIRST_M = {"zx": 0, "zy": 4, "xxB": 0, "yyB": 4}
_CS_LAYOUT = {"zx": (0, 32), "xxB": (32, 16), "yyB": (48, 16), "zy": (64, 32)}
_CS_NCH = 96


def _subregions(m):
    if m < 2:
        return [(8192, 4096, "ax", "zx"), (0, 2048, "xx", None),
                (2048, 2048, "xx", "xxB"), (4096, 4096, "xy", None)]
    if m < 4:
        return [(8192, 4096, "ax", "zx"), (2048, 2048, "xx", None),
                (4096, 4096, "xy", None)]
    if m < 6:
        return [(8192, 4096, "ay", "zy"), (4096, 2048, "yy", None),
                (6144, 2048, "yy", "yyB")]
    return [(8192, 4096, "ay", "zy"), (6144, 2048, "yy", None)]


# 2048-aligned chunks routed through the DVE trick-exp path (PSUM convert
# at 1x + fp16 row-sum at 4x); the rest take the ACT path.  Tuned so the
# ACT and DVE engine busies balance (~30.7K vs ~43K columns).
_DVE_CHUNKS = set()
for _m in range(4):
    _DVE_CHUNKS.add((_m, 4096))     # XY first half
    _DVE_CHUNKS.add((_m, 6144))     # XY second half
_DVE_CHUNKS.add((3, 2048))          # XX-R (m3)
for _m in (6, 7):
    _DVE_CHUNKS.add((_m, 6144))     # YY-R
for _m in (4, 5, 6, 7):
    _DVE_CHUNKS.add((_m, 8192))     # YZ first half

# Colacc adds routed to GpSimd (real HW rate ~2.0 ns/col vs 0.55 on
# VectorE): only a minority, scheduled early so the reduce-matmuls that
# consume the colaccs never wait on a GpSimd backlog.  Keyed by (m, cskey).
_GPS_ADD_KEYS = set()  # GPSIMD compute shares SBUF ports with DVE


def _act_tiling(width, first=False):
    """Split an ACT span into instruction widths (512-multiples)."""
    if first:
        # smaller leading instructions cut the startup bubble
        return [512, 512, 1024] + _act_tiling(width - 2048) if width > 2048 \
            else [512] * (width // 512)
    out = []
    while width:
        if width >= 2048:
            out.append(1536)
            width -= 1536
            if width == 512:
                out.append(512)
                width = 0
        else:
            out.append(min(width, 1536))
            width -= out[-1]
    return out


def _make_plan():
    """Emission-ordered op list per m-chunk.  Each entry:
    ('act', m, col0, w, slot, cls, cskey) or ('gps', m, col0, 2048, slot,
    cls, cskey).  Slot indices follow emission order."""
    plan = []
    slot = 0
    for m in range(8):
        ops = []
        for (r0, rw, cls, cskey) in _subregions(m):
            # split into GPS chunks and leftover ACT runs
            runs = []
            c = r0
            while c < r0 + rw:
                if (m, c) in _DVE_CHUNKS:
                    runs.append(("dve", c, 2048))
                    c += 2048
                else:
                    # extend an act run
                    if runs and runs[-1][0] == "act":
                        runs[-1] = ("act", runs[-1][1], runs[-1][2] + 2048)
                    else:
                        runs.append(("act", c, 2048))
                    c += 2048
            for kind, c0, w in runs:
                if kind == "dve":
                    ops.append(["dve", m, c0, 2048, None, cls, cskey])
                else:
                    for tw in _act_tiling(w):
                        ops.append(["act", m, c0, tw, None, cls, cskey])
                        c0 += tw
        for op in ops:
            op[4] = slot
            slot += 1
        plan.append([tuple(o) for o in ops])
    return plan, slot


_PLAN, _NSLOT = _make_plan()

# Reduce-emission points: after which m-chunk to emit each colacc's
# partition-sum matmuls (colacc writers: xxB m0-1, zx m0-3, yyB m4-5,
# zy m4-7).
_REDUCE_AFTER_M = {4: ["xxB"], 5: ["zx"], 7: ["yyB", "zy"]}


def _build_nc(T=1):
    import concourse.bacc as bacc
    import concourse.mybir as mybir
    import concourse.tile as tile

    f32 = mybir.dt.float32
    bf16 = mybir.dt.bfloat16

    nc = bacc.Bacc("TRN2")
    lhsT = nc.dram_tensor("lhsT", [128, 1024], bf16, kind="ExternalInput")
    rhsT = nc.dram_tensor("rhsT", [128, _N], bf16, kind="ExternalInput")
    out_s = nc.dram_tensor("out_s", [128, _NSLOT], f32, kind="ExternalOutput")
    out_cs = nc.dram_tensor("out_cs", [128, _CS_NCH], f32, kind="ExternalOutput")

    with tile.TileContext(nc) as tc:
        with (
            tc.tile_pool(name="singles", bufs=1) as singles,
            tc.tile_pool(name="etp", bufs=6) as etp,
            tc.tile_pool(name="i16p", bufs=4) as i16p,
            tc.tile_pool(name="et2p", bufs=4) as et2p,
            tc.tile_pool(name="psa", bufs=2, space="PSUM") as psa,
            tc.tile_pool(name="psd", bufs=2, space="PSUM") as psd,
        ):
            lhsT_t = singles.tile([128, 1024], bf16)
            rhsT_t = singles.tile([128, _N], bf16)
            ones_t = singles.tile([128, 1], bf16)
            act_warm = singles.tile([128, 1], f32)
            s_acc = singles.tile([128, _NSLOT], f32)
            colaccs = {k: singles.tile([128, w], bf16, name=f"colacc_{k}")
                       for k, w in _COLACC_SHAPE.items()}
            cs_sbuf = singles.tile([128, _CS_NCH], f32)

            nc.vector.memset(ones_t[:], 1.0)
            # Pull the exp ACT-table load into the input-DMA shadow.
            nc.scalar.activation(act_warm[:], ones_t[:],
                                 mybir.ActivationFunctionType.Exp, scale=1.0)
            # lhsT rides the GPSIMD SWDGE queue so it lands in parallel with
            # the rhs stream on the SP HWDGE queue.
            nc.gpsimd.dma_start(lhsT_t[:, 0:128], lhsT[:, 0:128])
            nc.sync.dma_start(rhsT_t[:, 0:1024], rhsT[:, 0:1024])
            nc.gpsimd.dma_start(lhsT_t[:, 128:1024], lhsT[:, 128:1024])
            nc.sync.dma_start(rhsT_t[:, 1024:2048], rhsT[:, 1024:2048])
            for p in range(1, _N // 2048):
                nc.sync.dma_start(rhsT_t[:, p * 2048:(p + 1) * 2048],
                                  rhsT[:, p * 2048:(p + 1) * 2048])

            for _t in range(T):
                _emit_body(nc, tc, etp, i16p, et2p, psa, psd, lhsT_t,
                           rhsT_t, ones_t, s_acc, colaccs, cs_sbuf, _t)

            nc.sync.dma_start(out_s[:], s_acc[:])
            nc.sync.dma_start(out_cs[:], cs_sbuf[:])

    nc.finalize()
    return nc


def _emit_body(nc, tc, etp, i16p, et2p, psa, psd, lhsT_t, rhsT_t,
               ones_t, s_acc, colaccs, cs_sbuf, t):
    import concourse.mybir as mybir

    f32 = mybir.dt.float32
    bf16 = mybir.dt.bfloat16
    fp16 = mybir.dt.float16
    i16 = mybir.dt.int16
    Exp = mybir.ActivationFunctionType.Exp
    mult = mybir.AluOpType.mult
    add = mybir.AluOpType.add

    def emit_reduce(key):
        base, nch = _CS_LAYOUT[key]
        cs_ps = psd.tile([128, 512], f32, tag="d", name=f"csps_{key}_{t}")
        for ch in range(nch):
            nc.tensor.matmul(
                cs_ps[:, ch:ch + 1],
                colaccs[key][:, ch * 128:(ch + 1) * 128],
                ones_t[:],
                start=True, stop=True,
            )
        nc.vector.tensor_copy(cs_sbuf[:, base:base + nch], cs_ps[:, 0:nch])

    for m, ops in enumerate(_PLAN):
        lhs_chunk = lhsT_t[:, m * 128:(m + 1) * 128]
        for (kind, _m, col0, width, slot, cls, cskey) in ops:
            first_writer = (cskey is not None
                            and m == _COLACC_FIRST_M[cskey])
            if kind == "act":
                pt = psa.tile([128, 1536], f32, tag="a",
                              name=f"pa_{t}_{m}_{col0}")
                for k in range(width // 512):
                    nc.tensor.matmul(
                        pt[:, k * 512:(k + 1) * 512],
                        lhs_chunk,
                        rhsT_t[:, col0 + k * 512:col0 + (k + 1) * 512],
                        start=True, stop=True,
                    )
                if first_writer:
                    off = col0 - _COLACC_BASE[cskey]
                    dst = colaccs[cskey][:, off:off + width]
                else:
                    et = etp.tile([128, 1536], bf16, tag="et",
                                  name=f"et_{t}_{m}_{col0}")
                    dst = et[:, 0:width]
                nc.scalar.activation(
                    dst, pt[:, 0:width], Exp, scale=1.0 / _TEMP,
                    accum_out=s_acc[:, slot:slot + 1],
                )
                if cskey is not None and not first_writer:
                    off = col0 - _COLACC_BASE[cskey]
                    ca = colaccs[cskey][:, off:off + width]
                    eng = (nc.gpsimd if (m, cskey) in _GPS_ADD_KEYS
                           else nc.vector)
                    eng.tensor_add(ca, ca, dst)
            else:
                # DVE path: int16 exp-bit convert straight from PSUM (1x),
                # then the 4x-mode fp16 row-sum tensor_scalar.
                it = i16p.tile([128, 2048], i16, tag="i16",
                               name=f"it_{t}_{m}_{col0}")
                for k in range(4):
                    pg = psd.tile([128, 512], f32, tag="d",
                                  name=f"pg_{t}_{m}_{col0}_{k}")
                    nc.tensor.matmul(
                        pg[:],
                        lhs_chunk,
                        rhsT_t[:, col0 + k * 512:col0 + (k + 1) * 512],
                        start=True, stop=True,
                    )
                    nc.vector.tensor_scalar(it[:, k * 512:(k + 1) * 512],
                                            pg[:], float(_EXPA),
                                            float(_EXPB), mult, add)
                if first_writer:
                    off = col0 - _COLACC_BASE[cskey]
                    dst = colaccs[cskey][:, off:off + width]
                else:
                    et2 = et2p.tile([128, 2048], bf16, tag="et2",
                                    name=f"e2_{t}_{m}_{col0}")
                    dst = et2[:]
                nc.vector.tensor_scalar(dst, it[:].bitcast(fp16), 1.0, None,
                                        mult, add,
                                        accum_out=s_acc[:, slot:slot + 1])
                if cskey is not None and not first_writer:
                    off = col0 - _COLACC_BASE[cskey]
                    ca = colaccs[cskey][:, off:off + width]
                    eng = (nc.gpsimd if (m, cskey) in _GPS_ADD_KEYS
                           else nc.vector)
                    eng.tensor_add(ca, ca, dst)
        for key in _REDUCE_AFTER_M.get(m, ()):
            emit_reduce(key)


class _Exec:
    """Cached sharded-jit executor for the finalized Bass module (modeled on
    concourse.bass2jax.run_bass_via_pjrt, but reusable across calls)."""

    def __init__(self, nc, n_cores):
        import jax
        import concourse.mybir as mybir
        from concourse import bass2jax
        from jax.sharding import Mesh, PartitionSpec
        from jax.experimental.shard_map import shard_map

        bass2jax.install_neuronx_cc_hook()
        self._jax = jax
        self.nc = nc
        self.n_cores = n_cores
        partition_name = (
            nc.partition_id_tensor.name if nc.partition_id_tensor else None
        )
        in_names, out_names, out_avals, zero_outs = [], [], [], []
        for alloc in nc.m.functions[0].allocations:
            if not isinstance(alloc, mybir.MemoryLocationSet):
                continue
            name = alloc.memorylocations[0].name
            if alloc.kind == "ExternalInput":
                if name != partition_name:
                    in_names.append(name)
            elif alloc.kind == "ExternalOutput":
                shape = tuple(alloc.tensor_shape)
                dtype = mybir.dt.np(alloc.dtype)
                out_names.append(name)
                out_avals.append(jax.core.ShapedArray(shape, dtype))
                zero_outs.append(np.zeros(shape, dtype))
        self.in_names = list(in_names)
        self.out_names = out_names
        self.out_avals = out_avals
        self.zero_outs = zero_outs
        n_params = len(in_names)
        n_outs = len(out_names)
        bind_in_names = in_names + out_names + (
            [partition_name] if partition_name else []
        )

        def _body(*args):
            operands = list(args)
            if partition_name is not None:
                operands.append(bass2jax.partition_id_tensor())
            outs = bass2jax._bass_exec_p.bind(
                *operands,
                out_avals=tuple(out_avals),
                in_names=tuple(bind_in_names),
                out_names=tuple(out_names),
                lowering_input_output_aliases=(),
                sim_require_finite=True,
                sim_require_nnan=True,
                nc=nc,
            )
            return tuple(outs)

        devices = jax.devices()[:n_cores]
        assert len(devices) == n_cores
        self.mesh = Mesh(np.asarray(devices), ("core",))
        donate = tuple(range(n_params, n_params + n_outs))
        self.fn = jax.jit(
            shard_map(
                _body,
                mesh=self.mesh,
                in_specs=(PartitionSpec("core"),) * (n_params + n_outs),
                out_specs=(PartitionSpec("core"),) * n_outs,
                check_rep=False,
            ),
            donate_argnums=donate,
            keep_unused=True,
        )

    def make_zeros(self):
        return [
            np.zeros((self.n_cores * z.shape[0], *z.shape[1:]), z.dtype)
            for z in self.zero_outs
        ]

    def concat_inputs(self, in_maps):
        return [
            np.concatenate([np.asarray(in_maps[c][n]) for c in range(self.n_cores)], axis=0)
            for n in self.in_names
        ]

    def run_raw(self, concat_in, zeros):
        return self.fn(*concat_in, *zeros)

    def __call__(self, in_maps):
        out_arrs = self.fn(*self.concat_inputs(in_maps), *self.make_zeros())
        res = []
        for c in range(self.n_cores):
            res.append({
                name: np.asarray(out_arrs[i]).reshape(
                    self.n_cores, *self.out_avals[i].shape)[c]
                for i, name in enumerate(self.out_names)
            })
        return res


def _get_exec(T=1):
    key = ("exec", T)
    if key not in _STATE:
        nc = _build_nc(T)
        _STATE[key] = _Exec(nc, _NCORES)
    return _STATE[key]


def _mlod_exact(s, d):
    """mean_{ij} log(s[i] - d[j]) computed directly (chunked)."""
    tot = 0.0
    for i0 in range(0, s.shape[0], 256):
        tot += float(np.log(np.subtract.outer(s[i0:i0 + 256], d)).sum())
    return tot / (s.shape[0] * d.shape[0])


def _mlod(s, d):
    """mean_{ij} log(s[i] - d[j]) via binomial power-series factorization.

    log(s_i - d_j) = log M + log1p(u_i - v_j) with M = mean(s) - mean(d),
    u = (s-mean(s))/M, v = (d-mean(d))/M.  mean_{ij} (u_i-v_j)^k factorizes
    into products of power means, so the double mean is O(B*K).
    """
    from math import comb

    s = np.asarray(s, np.float64)
    d = np.asarray(d, np.float64)
    ms, md = s.mean(), d.mean()
    M = ms - md
    if not np.isfinite(M) or M <= 0:
        return _mlod_exact(s, d)
    u = (s - ms) / M
    v = (d - md) / M
    wmax = np.abs(u).max() + np.abs(v).max()
    if wmax > 0.5:
        return _mlod_exact(s, d)
    K = 120
    P = np.empty(K + 1)
    Q = np.empty(K + 1)
    up = np.ones_like(u)
    vp = np.ones_like(v)
    for k in range(K + 1):
        P[k] = up.mean()
        Q[k] = vp.mean()
        up *= u
        vp *= -v
    total = 0.0
    for k in range(1, K + 1):
        mk = 0.0
        for m in range(k + 1):
            mk += comb(k, m) * P[m] * Q[k - m]
        term = (1.0 if k % 2 == 1 else -1.0) / k * mk
        total += term
        if k > 6 and abs(term) < 1e-18 * max(1.0, abs(total)):
            break
    return float(np.log(M)) + total


def _host_prepare(x):
    """fp32 normalize (mirrors reference), bf16 cast, per-core device inputs."""
    x = np.asarray(x, np.float32)
    n = np.sqrt((x * x).sum(axis=1, keepdims=True))
    xn = x / np.maximum(n, _EPS)
    xnb = xn.astype(_BF16)
    rhsT = np.ascontiguousarray(xnb.T)  # [128, 12288]
    H = _B // 2
    in_maps = []
    for c in range(_NCORES):
        lo = c * 256
        rows = np.concatenate([
            xnb[lo:lo + 256],                    # low x  (m0, m1)
            xnb[H + lo:H + lo + 256],            # high x (m2, m3)
            xnb[_B + lo:_B + lo + 256],          # low y  (m4, m5)
            xnb[_B + H + lo:_B + H + lo + 256],  # high y (m6, m7)
        ], axis=0)
        in_maps.append({"lhsT": np.ascontiguousarray(rows.T), "rhsT": rhsT})
    return xn, in_maps


_TARGET_VEC = {"xx": 0, "xy": 1, "ax": 2, "yy": 3, "ay": 4}


def _assemble_s(results):
    """Decode device outputs into the seven s vectors (fp64)."""
    H = _B // 2
    vecs = [np.zeros(_B) for _ in range(5)]  # xx, xy, ax, yy, ay
    s_zx = np.zeros(_B)
    s_zy = np.zeros(_B)
    for c in range(_NCORES):
        sa = np.asarray(results[c]["out_s"], np.float64)  # [128, NSLOT]
        for m, ops in enumerate(_PLAN):
            half = (m // 2) % 2            # 0 = low rows, 1 = high rows
            i0 = half * H + c * 256 + (m % 2) * 128
            for (kind, _m, col0, width, slot, cls, cskey) in ops:
                vecs[_TARGET_VEC[cls]][i0:i0 + 128] += sa[:, slot]
    # Column-sum contributions.
    cs_sum = np.zeros((128, _CS_NCH), np.float64)
    for c in range(_NCORES):
        cs_sum += np.asarray(results[c]["out_cs"], np.float64)
    # col idx base+ch holds colsums for accumulator column ch*128 + p
    s_zx += cs_sum[:, 0:32].T.reshape(-1)
    vecs[0][H:] += cs_sum[:, 32:48].T.reshape(-1)   # xx high-left
    vecs[3][H:] += cs_sum[:, 48:64].T.reshape(-1)   # yy high-left
    s_zy += cs_sum[:, 64:96].T.reshape(-1)
    s_xx, s_xy, s_ax, s_yy, s_ay = vecs
    return s_xx, s_xy, s_ax, s_yy, s_ay, s_zx, s_zy


def _host_combine(xn, results):
    xe = xn[:_B].astype(np.float64)
    ye = xn[_B:2 * _B].astype(np.float64)
    ze = xn[2 * _B:].astype(np.float64)
    inv_t = 1.0 / _TEMP
    d_xx = np.exp((xe * xe).sum(1) * inv_t)
    d_yy = np.exp((ye * ye).sum(1) * inv_t)
    d_xy = np.exp((xe * ye).sum(1) * inv_t)
    d_ax = np.exp((xe * ze).sum(1) * inv_t)
    d_ay = np.exp((ye * ze).sum(1) * inv_t)

    s_xx, s_xy, s_ax, s_yy, s_ay, s_zx, s_zy = _assemble_s(results)

    S_mut = s_xy + s_xx + s_yy
    D_mut = d_xy + d_xx + d_yy
    loss_mutual = -2.0 * float(np.log(d_xy).mean()) + 2.0 * _mlod(S_mut, D_mut)

    def aux(d, s):
        return -float(np.log(d).mean()) + _mlod(s, d)

    loss = (loss_mutual + aux(d_ax, s_ax) + aux(d_ay, s_ay)
            + aux(d_ax, s_zx) + aux(d_ay, s_zy))
    return np.array(loss, dtype=np.float32)


def kernel(x):
    ex = _get_exec()
    xn, in_maps = _host_prepare(x)
    results = ex(in_maps)
    return _host_combine(xn, results)


if __name__ == "__main__":
    rng = np.random.default_rng(0)
    x = rng.standard_normal((_N, _D)).astype(np.float32)
    print(kernel(x))


# revision 13
# speedup vs baseline: 1.1752x; 1.0327x over previous
"""Trainium2 Bass kernel for nn_LossNet_42494406426743 (contrastive loss_fn).

Math (reference, temp=0.1, B=4096):
    xn = l2_normalize(x); xe, ye, ze = split(xn, 3)
    For pairs (a,b) in {xx, yy, xy, xz, yz(+transposes zx, zy)}:
        d_ab[i] = exp(a_i.b_i/t)  (diagonal)
        s_ab[i] = sum_j exp(a_i.b_j/t)  (row sums of the exp-similarity matrix)
    loss = mean_{ij}[-2 log(d_xy[j]/((S[i]-D[j])))] + 4 aux terms of
           mean_{ij}[-log(d[j]/(s[i]-d[j]))]

Device work (sharded 8 ways over rows; each core owns 256 "low" + 256 "high"
rows of each of xe and ye; z never appears as a row operand).  The exp +
row-sum work (4.5*B^2/8 elements per core) is spread over FOUR engines:

  * TensorE: bf16 matmuls (stationary own-row chunks vs the SBUF-resident
    embedding matrix) into two PSUM rings, plus ones-matmul partition
    reductions of the column accumulators.
  * ScalarE (ACT ring, 2x1536 PSUM): exact exp via LUT with fused accum_out
    row-sums.
  * GpSimdE (GPS ring, 2x512 PSUM): evacuates the other matmul outputs to
    SBUF as bf16 logits (otherwise idle engine).
  * VectorE: approximate exp on the GPS-path logits via the fp16 bit trick
    -- tensor_scalar int16(logit*14773.13 + 15301.5) runs at 4x mode from
    bf16, and a second 4x tensor_scalar over the fp16-bitcast tile yields
    the row sum through accum_out.  Per-element error <4% and zero-mean
    (offset calibrated), so row sums over 4096 terms are accurate to ~1e-4.
    VectorE also accumulates the exp tiles of XZ^T / YZ^T (and the
    symmetric-block right halves) into column accumulators, whose
    partition sums recover the zx / zy row sums and the xx / yy high-row
    left halves without recomputing transposed exps.

Host work (O(B), fp64): diagonals, assembling s vectors, and the
mean_{ij} log(s[i]-d[j]) terms evaluated exactly via a binomial power-series
factorization (O(B*K) instead of O(B^2); exact fallback if out of range).
"""

import numpy as np
import ml_dtypes

_BF16 = ml_dtypes.bfloat16

# Problem constants (hardcoded per harness contract).
_N = 12288          # total rows
_D = 128            # feature dim
_B = 4096           # rows per split
_NCORES = 8
_TEMP = 0.1
_EPS = 1e-12

# fp16 bit-trick exp constants: int16(logit*EXPA + EXPB) viewed as fp16
# approximates exp(logit/temp).  EXPB includes a -58.5 offset that nulls the
# mean relative error of the linear-mantissa approximation.
_EXPA = 1024.0 * (1.0 / _TEMP) * np.log2(np.e)   # 14773.13
_EXPB = 15.0 * 1024.0 - 58.5

_STATE = {}

# --------------------------------------------------------------------------
# Static work plan.
# m-chunks: m0,m1 = "low" x rows, m2,m3 = "high" x rows, m4,m5 = low y,
# m6,m7 = high y (128 rows each).  Low rows compute their symmetric block
# fully; high rows compute only the right half and recover the left half
# from transposed colsums (xxB / yyB).
#
# Subregions per m-chunk (uniform target vector and colacc-ness), emitted
# z-first so the column accumulators complete early:
#   m0/m1: XZ[8192:12288|zx] XX-L[0:2048] xxB[2048:4096|xxB] XY[4096:8192]
#   m2/m3: XZ[8192:12288|zx] XX-R[2048:4096] XY[4096:8192]
#   m4/m5: YZ[8192:12288|zy] YY-L[4096:6144] yyB[6144:8192|yyB]
#   m6/m7: YZ[8192:12288|zy] YY-R[6144:8192]
_COLACC_SHAPE = {"zx": _B, "zy": _B, "xxB": 2048, "yyB": 2048}
_COLACC_BASE = {"zx": 8192, "zy": 8192, "xxB": 2048, "yyB": 6144}
_COLACC_FIRST_M = {"zx": 0, "zy": 4, "xxB": 0, "yyB": 4}
_CS_LAYOUT = {"zx": (0, 32), "xxB": (32, 16), "yyB": (48, 16), "zy": (64, 32)}
_CS_NCH = 96


def _subregions(m):
    if m < 2:
        return [(8192, 4096, "ax", "zx"), (0, 2048, "xx", None),
                (2048, 2048, "xx", "xxB"), (4096, 4096, "xy", None)]
    if m < 4:
        return [(8192, 4096, "ax", "zx"), (2048, 2048, "xx", None),
                (4096, 4096, "xy", None)]
    if m < 6:
        return [(8192, 4096, "ay", "zy"), (4096, 2048, "yy", None),
                (6144, 2048, "yy", "yyB")]
    return [(8192, 4096, "ay", "zy"), (6144, 2048, "yy", None)]


# 2048-aligned chunks routed through the DVE trick-exp path (PSUM convert
# at 1x + fp16 row-sum at 4x); the rest take the ACT path.  Tuned so the
# ACT and DVE engine busies balance (~30.7K vs ~43K columns).
_DVE_CHUNKS = set()
for _m in range(4):
    _DVE_CHUNKS.add((_m, 4096))     # XY first half
for _m in (6, 7):
    _DVE_CHUNKS.add((_m, 6144))     # YY-R

# Colacc adds routed to GpSimd (real HW rate ~2.0 ns/col vs 0.55 on
# VectorE): only a minority, scheduled early so the reduce-matmuls that
# consume the colaccs never wait on a GpSimd backlog.  Keyed by (m, cskey).
_GPS_ADD_KEYS = set()  # GPSIMD compute shares SBUF ports with DVE


def _act_tiling(width, first=False):
    """Split an ACT span into instruction widths (2048-wide; the first
    instruction of the kernel is split in half to cut the startup bubble)."""
    out = [1024, 1024] if first else []
    if first:
        width -= 2048
    out += [2048] * (width // 2048)
    return out


def _make_plan():
    """Emission-ordered op list per m-chunk.  Each entry:
    ('act', m, col0, w, slot, cls, cskey) or ('gps', m, col0, 2048, slot,
    cls, cskey).  Slot indices follow emission order."""
    plan = []
    slot = 0
    for m in range(8):
        ops = []
        for (r0, rw, cls, cskey) in _subregions(m):
            # split into GPS chunks and leftover ACT runs
            runs = []
            c = r0
            while c < r0 + rw:
                if (m, c) in _DVE_CHUNKS:
                    runs.append(("dve", c, 2048))
                    c += 2048
                else:
                    # extend an act run
                    if runs and runs[-1][0] == "act":
                        runs[-1] = ("act", runs[-1][1], runs[-1][2] + 2048)
                    else:
                        runs.append(("act", c, 2048))
                    c += 2048
            for kind, c0, w in runs:
                if kind == "dve":
                    ops.append(["dve", m, c0, 2048, None, cls, cskey])
                else:
                    first = (m == 0 and not ops)
                    for tw in _act_tiling(w, first=first):
                        ops.append(["act", m, c0, tw, None, cls, cskey])
                        c0 += tw
        ops = [o for o in ops if o[0] == "act"] + \
              [o for o in ops if o[0] == "dve"]
        for op in ops:
            op[4] = slot
            slot += 1
        plan.append([tuple(o) for o in ops])
    return plan, slot


_PLAN, _NSLOT = _make_plan()

# Reduce-emission points: after which m-chunk to emit each colacc's
# partition-sum matmuls (colacc writers: xxB m0-1, zx m0-3, yyB m4-5,
# zy m4-7).
_REDUCE_AFTER_M = {4: ["xxB"], 5: ["zx"], 7: ["yyB", "zy"]}


def _build_nc(T=1):
    import concourse.bacc as bacc
    import concourse.mybir as mybir
    import concourse.tile as tile

    f32 = mybir.dt.float32
    bf16 = mybir.dt.bfloat16

    nc = bacc.Bacc("TRN2")
    lhsT = nc.dram_tensor("lhsT", [128, 1024], bf16, kind="ExternalInput")
    rhsT = nc.dram_tensor("rhsT", [128, _N], bf16, kind="ExternalInput")
    out_s = nc.dram_tensor("out_s", [128, _NSLOT], f32, kind="ExternalOutput")
    out_cs = nc.dram_tensor("out_cs", [128, _CS_NCH], f32, kind="ExternalOutput")

    with tile.TileContext(nc) as tc:
        with (
            tc.tile_pool(name="singles", bufs=1) as singles,
            tc.tile_pool(name="etp", bufs=6) as etp,
            tc.tile_pool(name="i16p", bufs=4) as i16p,
            tc.tile_pool(name="et2p", bufs=4) as et2p,
            tc.tile_pool(name="psa", bufs=2, space="PSUM") as psa,
        ):
            lhsT_t = singles.tile([128, 1024], bf16)
            rhsT_t = singles.tile([128, _N], bf16)
            ones_t = singles.tile([128, 1], bf16)
            act_warm = singles.tile([128, 1], f32)
            s_acc = singles.tile([128, _NSLOT], f32)
            colaccs = {k: singles.tile([128, w], bf16, name=f"colacc_{k}")
                       for k, w in _COLACC_SHAPE.items()}
            cs_sbuf = singles.tile([128, _CS_NCH], f32)

            nc.vector.memset(ones_t[:], 1.0)
            # Pull the exp ACT-table load into the input-DMA shadow.
            nc.scalar.activation(act_warm[:], ones_t[:],
                                 mybir.ActivationFunctionType.Exp, scale=1.0)
            # lhsT rides the GPSIMD SWDGE queue so it lands in parallel with
            # the rhs stream on the SP HWDGE queue.
            nc.gpsimd.dma_start(lhsT_t[:, 0:128], lhsT[:, 0:128])
            nc.sync.dma_start(rhsT_t[:, 0:1024], rhsT[:, 0:1024])
            nc.gpsimd.dma_start(lhsT_t[:, 128:1024], lhsT[:, 128:1024])
            nc.sync.dma_start(rhsT_t[:, 1024:2048], rhsT[:, 1024:2048])
            for p in range(1, _N // 2048):
                nc.sync.dma_start(rhsT_t[:, p * 2048:(p + 1) * 2048],
                                  rhsT[:, p * 2048:(p + 1) * 2048])

            for _t in range(T):
                _emit_body(nc, tc, etp, i16p, et2p, psa, lhsT_t,
                           rhsT_t, ones_t, s_acc, colaccs, cs_sbuf, _t)

            nc.sync.dma_start(out_s[:], s_acc[:])
            nc.sync.dma_start(out_cs[:], cs_sbuf[:])

    nc.finalize()
    return nc


def _emit_body(nc, tc, etp, i16p, et2p, psa, lhsT_t, rhsT_t,
               ones_t, s_acc, colaccs, cs_sbuf, t):
    import concourse.mybir as mybir

    f32 = mybir.dt.float32
    bf16 = mybir.dt.bfloat16
    fp16 = mybir.dt.float16
    i16 = mybir.dt.int16
    Exp = mybir.ActivationFunctionType.Exp
    mult = mybir.AluOpType.mult
    add = mybir.AluOpType.add

    def emit_reduce(key):
        base, nch = _CS_LAYOUT[key]
        cs_ps = psa.tile([128, 2048], f32, tag="a", name=f"csps_{key}_{t}")
        for ch in range(nch):
            nc.tensor.matmul(
                cs_ps[:, ch:ch + 1],
                colaccs[key][:, ch * 128:(ch + 1) * 128],
                ones_t[:],
                start=True, stop=True,
            )
        nc.vector.tensor_copy(cs_sbuf[:, base:base + nch], cs_ps[:, 0:nch])

    for m, ops in enumerate(_PLAN):
        lhs_chunk = lhsT_t[:, m * 128:(m + 1) * 128]
        for (kind, _m, col0, width, slot, cls, cskey) in ops:
            first_writer = (cskey is not None
                            and m == _COLACC_FIRST_M[cskey])
            if kind == "act":
                pt = psa.tile([128, 2048], f32, tag="a",
                              name=f"pa_{t}_{m}_{col0}")
                for k in range(width // 512):
                    nc.tensor.matmul(
                        pt[:, k * 512:(k + 1) * 512],
                        lhs_chunk,
                        rhsT_t[:, col0 + k * 512:col0 + (k + 1) * 512],
                        start=True, stop=True,
                    )
                if first_writer:
                    off = col0 - _COLACC_BASE[cskey]
                    dst = colaccs[cskey][:, off:off + width]
                else:
                    et = etp.tile([128, 2048], bf16, tag="et",
                                  name=f"et_{t}_{m}_{col0}")
                    dst = et[:, 0:width]
                nc.scalar.activation(
                    dst, pt[:, 0:width], Exp, scale=1.0 / _TEMP,
                    accum_out=s_acc[:, slot:slot + 1],
                )
                if cskey is not None and not first_writer:
                    off = col0 - _COLACC_BASE[cskey]
                    ca = colaccs[cskey][:, off:off + width]
                    eng = (nc.gpsimd if (m, cskey) in _GPS_ADD_KEYS
                           else nc.vector)
                    eng.tensor_add(ca, ca, dst)
            else:
                # DVE path: int16 exp-bit convert straight from PSUM (1x),
                # then the 4x-mode fp16 row-sum tensor_scalar.
                it = i16p.tile([128, 2048], i16, tag="i16",
                               name=f"it_{t}_{m}_{col0}")
                pg = psa.tile([128, 2048], f32, tag="a",
                              name=f"pg_{t}_{m}_{col0}")
                for k in range(4):
                    nc.tensor.matmul(
                        pg[:, k * 512:(k + 1) * 512],
                        lhs_chunk,
                        rhsT_t[:, col0 + k * 512:col0 + (k + 1) * 512],
                        start=True, stop=True,
                    )
                nc.vector.tensor_scalar(it[:], pg[:], float(_EXPA),
                                        float(_EXPB), mult, add)
                if first_writer:
                    off = col0 - _COLACC_BASE[cskey]
                    dst = colaccs[cskey][:, off:off + width]
                else:
                    et2 = et2p.tile([128, 2048], bf16, tag="et2",
                                    name=f"e2_{t}_{m}_{col0}")
                    dst = et2[:]
                nc.vector.tensor_scalar(dst, it[:].bitcast(fp16), 1.0, None,
                                        mult, add,
                                        accum_out=s_acc[:, slot:slot + 1])
                if cskey is not None and not first_writer:
                    off = col0 - _COLACC_BASE[cskey]
                    ca = colaccs[cskey][:, off:off + width]
                    eng = (nc.gpsimd if (m, cskey) in _GPS_ADD_KEYS
                           else nc.vector)
                    eng.tensor_add(ca, ca, dst)
        for key in _REDUCE_AFTER_M.get(m, ()):
            emit_reduce(key)


class _Exec:
    """Cached sharded-jit executor for the finalized Bass module (modeled on
    concourse.bass2jax.run_bass_via_pjrt, but reusable across calls)."""

    def __init__(self, nc, n_cores):
        import jax
        import concourse.mybir as mybir
        from concourse import bass2jax
        from jax.sharding import Mesh, PartitionSpec
        from jax.experimental.shard_map import shard_map

        bass2jax.install_neuronx_cc_hook()
        self._jax = jax
        self.nc = nc
        self.n_cores = n_cores
        partition_name = (
            nc.partition_id_tensor.name if nc.partition_id_tensor else None
        )
        in_names, out_names, out_avals, zero_outs = [], [], [], []
        for alloc in nc.m.functions[0].allocations:
            if not isinstance(alloc, mybir.MemoryLocationSet):
                continue
            name = alloc.memorylocations[0].name
            if alloc.kind == "ExternalInput":
                if name != partition_name:
                    in_names.append(name)
            elif alloc.kind == "ExternalOutput":
                shape = tuple(alloc.tensor_shape)
                dtype = mybir.dt.np(alloc.dtype)
                out_names.append(name)
                out_avals.append(jax.core.ShapedArray(shape, dtype))
                zero_outs.append(np.zeros(shape, dtype))
        self.in_names = list(in_names)
        self.out_names = out_names
        self.out_avals = out_avals
        self.zero_outs = zero_outs
        n_params = len(in_names)
        n_outs = len(out_names)
        bind_in_names = in_names + out_names + (
            [partition_name] if partition_name else []
        )

        def _body(*args):
            operands = list(args)
            if partition_name is not None:
                operands.append(bass2jax.partition_id_tensor())
            outs = bass2jax._bass_exec_p.bind(
                *operands,
                out_avals=tuple(out_avals),
                in_names=tuple(bind_in_names),
                out_names=tuple(out_names),
                lowering_input_output_aliases=(),
                sim_require_finite=True,
                sim_require_nnan=True,
                nc=nc,
            )
            return tuple(outs)

        devices = jax.devices()[:n_cores]
        assert len(devices) == n_cores
        self.mesh = Mesh(np.asarray(devices), ("core",))
        donate = tuple(range(n_params, n_params + n_outs))
        self.fn = jax.jit(
            shard_map(
                _body,
                mesh=self.mesh,
                in_specs=(PartitionSpec("core"),) * (n_params + n_outs),
                out_specs=(PartitionSpec("core"),) * n_outs,
                check_rep=False,
            ),
            donate_argnums=donate,
            keep_unused=True,
        )

    def make_zeros(self):
        return [
            np.zeros((self.n_cores * z.shape[0], *z.shape[1:]), z.dtype)
            for z in self.zero_outs
        ]

    def concat_inputs(self, in_maps):
        return [
            np.concatenate([np.asarray(in_maps[c][n]) for c in range(self.n_cores)], axis=0)
            for n in self.in_names
        ]

    def run_raw(self, concat_in, zeros):
        return self.fn(*concat_in, *zeros)

    def __call__(self, in_maps):
        out_arrs = self.fn(*self.concat_inputs(in_maps), *self.make_zeros())
        res = []
        for c in range(self.n_cores):
            res.append({
                name: np.asarray(out_arrs[i]).reshape(
                    self.n_cores, *self.out_avals[i].shape)[c]
                for i, name in enumerate(self.out_names)
            })
        return res


def _get_exec(T=1):
    key = ("exec", T)
    if key not in _STATE:
        nc = _build_nc(T)
        _STATE[key] = _Exec(nc, _NCORES)
    return _STATE[key]


def _mlod_exact(s, d):
    """mean_{ij} log(s[i] - d[j]) computed directly (chunked)."""
    tot = 0.0
    for i0 in range(0, s.shape[0], 256):
        tot += float(np.log(np.subtract.outer(s[i0:i0 + 256], d)).sum())
    return tot / (s.shape[0] * d.shape[0])


def _mlod(s, d):
    """mean_{ij} log(s[i] - d[j]) via binomial power-series factorization.

    log(s_i - d_j) = log M + log1p(u_i - v_j) with M = mean(s) - mean(d),
    u = (s-mean(s))/M, v = (d-mean(d))/M.  mean_{ij} (u_i-v_j)^k factorizes
    into products of power means, so the double mean is O(B*K).
    """
    from math import comb

    s = np.asarray(s, np.float64)
    d = np.asarray(d, np.float64)
    ms, md = s.mean(), d.mean()
    M = ms - md
    if not np.isfinite(M) or M <= 0:
        return _mlod_exact(s, d)
    u = (s - ms) / M
    v = (d - md) / M
    wmax = np.abs(u).max() + np.abs(v).max()
    if wmax > 0.5:
        return _mlod_exact(s, d)
    K = 120
    P = np.empty(K + 1)
    Q = np.empty(K + 1)
    up = np.ones_like(u)
    vp = np.ones_like(v)
    for k in range(K + 1):
        P[k] = up.mean()
        Q[k] = vp.mean()
        up *= u
        vp *= -v
    total = 0.0
    for k in range(1, K + 1):
        mk = 0.0
        for m in range(k + 1):
            mk += comb(k, m) * P[m] * Q[k - m]
        term = (1.0 if k % 2 == 1 else -1.0) / k * mk
        total += term
        if k > 6 and abs(term) < 1e-18 * max(1.0, abs(total)):
            break
    return float(np.log(M)) + total


def _host_prepare(x):
    """fp32 normalize (mirrors reference), bf16 cast, per-core device inputs."""
    x = np.asarray(x, np.float32)
    n = np.sqrt((x * x).sum(axis=1, keepdims=True))
    xn = x / np.maximum(n, _EPS)
    xnb = xn.astype(_BF16)
    rhsT = np.ascontiguousarray(xnb.T)  # [128, 12288]
    H = _B // 2
    in_maps = []
    for c in range(_NCORES):
        lo = c * 256
        rows = np.concatenate([
            xnb[lo:lo + 256],                    # low x  (m0, m1)
            xnb[H + lo:H + lo + 256],            # high x (m2, m3)
            xnb[_B + lo:_B + lo + 256],          # low y  (m4, m5)
            xnb[_B + H + lo:_B + H + lo + 256],  # high y (m6, m7)
        ], axis=0)
        in_maps.append({"lhsT": np.ascontiguousarray(rows.T), "rhsT": rhsT})
    return xn, in_maps


_TARGET_VEC = {"xx": 0, "xy": 1, "ax": 2, "yy": 3, "ay": 4}


def _assemble_s(results):
    """Decode device outputs into the seven s vectors (fp64)."""
    H = _B // 2
    vecs = [np.zeros(_B) for _ in range(5)]  # xx, xy, ax, yy, ay
    s_zx = np.zeros(_B)
    s_zy = np.zeros(_B)
    for c in range(_NCORES):
        sa = np.asarray(results[c]["out_s"], np.float64)  # [128, NSLOT]
        for m, ops in enumerate(_PLAN):
            half = (m // 2) % 2            # 0 = low rows, 1 = high rows
            i0 = half * H + c * 256 + (m % 2) * 128
            for (kind, _m, col0, width, slot, cls, cskey) in ops:
                vecs[_TARGET_VEC[cls]][i0:i0 + 128] += sa[:, slot]
    # Column-sum contributions.
    cs_sum = np.zeros((128, _CS_NCH), np.float64)
    for c in range(_NCORES):
        cs_sum += np.asarray(results[c]["out_cs"], np.float64)
    # col idx base+ch holds colsums for accumulator column ch*128 + p
    s_zx += cs_sum[:, 0:32].T.reshape(-1)
    vecs[0][H:] += cs_sum[:, 32:48].T.reshape(-1)   # xx high-left
    vecs[3][H:] += cs_sum[:, 48:64].T.reshape(-1)   # yy high-left
    s_zy += cs_sum[:, 64:96].T.reshape(-1)
    s_xx, s_xy, s_ax, s_yy, s_ay = vecs
    return s_xx, s_xy, s_ax, s_yy, s_ay, s_zx, s_zy


def _host_combine(xn, results):
    xe = xn[:_B].astype(np.float64)
    ye = xn[_B:2 * _B].astype(np.float64)
    ze = xn[2 * _B:].astype(np.float64)
    inv_t = 1.0 / _TEMP
    d_xx = np.exp((xe * xe).sum(1) * inv_t)
    d_yy = np.exp((ye * ye).sum(1) * inv_t)
    d_xy = np.exp((xe * ye).sum(1) * inv_t)
    d_ax = np.exp((xe * ze).sum(1) * inv_t)
    d_ay = np.exp((ye * ze).sum(1) * inv_t)

    s_xx, s_xy, s_ax, s_yy, s_ay, s_zx, s_zy = _assemble_s(results)

    S_mut = s_xy + s_xx + s_yy
    D_mut = d_xy + d_xx + d_yy
    loss_mutual = -2.0 * float(np.log(d_xy).mean()) + 2.0 * _mlod(S_mut, D_mut)

    def aux(d, s):
        return -float(np.log(d).mean()) + _mlod(s, d)

    loss = (loss_mutual + aux(d_ax, s_ax) + aux(d_ay, s_ay)
            + aux(d_ax, s_zx) + aux(d_ay, s_zy))
    return np.array(loss, dtype=np.float32)


def kernel(x):
    ex = _get_exec()
    xn, in_maps = _host_prepare(x)
    results = ex(in_maps)
    return _host_combine(xn, results)


if __name__ == "__main__":
    rng = np.random.default_rng(0)
    x = rng.standard_normal((_N, _D)).astype(np.float32)
    print(kernel(x))
